# revision 6
# baseline (speedup 1.0000x reference)
"""Trainium2 Bass kernel for a 2-layer Keras-style GRU encoder (reset_after=True).

Problem: x [64, 256, 1024] fp32 -> 2x GRU(1024) -> (output [64, 256, 1024], state [64, 1024]).

Strategy (8 NeuronCores, data-parallel over batch, B_local=8 per core, no
collectives; weights replicated in bf16):

  All per-step tensors live in a transposed layout: units on SBUF partitions,
  (time*batch) on the free dimension. The recurrent matmul h @ U runs with U
  tiles as the stationary operand (out = U_tile.T @ h^T), so each step is
  24 Mtiles x 8 Ktiles = 192 LDWEIGHTS+MATMUL pairs with an 8-wide moving
  operand -- bound by the bf16 fast-weight-load path (~46 ns/pair measured).
  Gate elementwise (sigmoid/tanh/blend) runs on ACT+DVE in [128, 64..128]
  tiles and hides entirely under the PE stream of the next step.

  Phases (all SBUF-resident, big regions reused across phases via pool tags):
    A: load U1/biases, transpose x into x^T (PE transpose-mode)
    B: xp1 = W1.T-tiled @ x^T (+ folded biases) -> bf16, W1 streamed from HBM
    C: 256 GRU steps of layer 1, h history accumulates as y1^T in SBUF
    D: xp2 = W2 @ y1^T (+ biases); U2 load overlaps
    E: 256 GRU steps of layer 2; every 16 steps transpose the y2^T chunk back
       to row-major and DMA out; final hidden state from the last chunk.

  bf16 matmuls with fp32 PSUM accumulation and fp32 gate arithmetic give
  ~7e-3 max relative error vs the fp32 reference (validated by simulation).
"""
import os
import numpy as np
import ml_dtypes

import concourse.bass as bass
import concourse.tile as tile
from concourse import bacc, mybir
from concourse.bass_utils import run_bass_kernel_spmd
from concourse.masks import make_identity

F32 = mybir.dt.float32
BF16 = mybir.dt.bfloat16
AF = mybir.ActivationFunctionType
OP = mybir.AluOpType

P = 128          # SBUF partitions
NCORES = 8
B, T, F, UN = 64, int(os.environ.get("GRU_T", "256")), 1024, 1024
BL = B // NCORES          # batch per core
G3 = 3 * UN               # gate dim
KT = F // P               # K tiles (contraction)
MT = G3 // P              # M tiles (gate units)
CH = 16                   # steps per output chunk
NCH = T // CH             # chunks
ROWS = T * BL             # global rows (t-major: row = t*BL + b)
CROWS = CH * BL           # rows per chunk (=128)
N4 = 4                    # 512-wide row slabs in the xp matmuls
NW = ROWS // N4           # 512

_CACHED = {}


def _build():
    nc = bacc.Bacc("TRN2", target_bir_lowering=False, debug=False, num_devices=NCORES)

    x_d = nc.dram_tensor("x", [BL, T, F], F32, kind="ExternalInput")
    hid_d = nc.dram_tensor("hidden", [BL, UN], F32, kind="ExternalInput")
    u1_d = nc.dram_tensor("u1", [UN, G3], BF16, kind="ExternalInput")
    u2_d = nc.dram_tensor("u2", [UN, G3], BF16, kind="ExternalInput")
    w1_d = nc.dram_tensor("w1t", [KT * MT, P, P], BF16, kind="ExternalInput")
    w2_d = nc.dram_tensor("w2t", [KT * MT, P, P], BF16, kind="ExternalInput")
    bx1_d = nc.dram_tensor("bx1", [P, MT], F32, kind="ExternalInput")
    bx2_d = nc.dram_tensor("bx2", [P, MT], F32, kind="ExternalInput")
    bh1_d = nc.dram_tensor("bh1", [P, KT * BL], F32, kind="ExternalInput")
    bh2_d = nc.dram_tensor("bh2", [P, KT * BL], F32, kind="ExternalInput")
    y_d = nc.dram_tensor("y", [BL, T, UN], F32, kind="ExternalOutput")
    hl_d = nc.dram_tensor("hlast", [BL, UN], F32, kind="ExternalOutput")

    # (t b)-row views of x / y: [chunk][t'][b][feat]
    xsrc = x_d.ap().rearrange("b (c t) f -> c t b f", c=NCH)
    ydst = y_d.ap().rearrange("b (c t) u -> c t b u", c=NCH)

    with tile.TileContext(nc) as tc:
        with (
            tc.tile_pool(name="resA", bufs=1) as resA,   # ubig: U1 then U2
            tc.tile_pool(name="resB", bufs=1) as resB,   # xpbig: xp1 then xp2
            tc.tile_pool(name="resC", bufs=1) as resC,   # rbig: x^T then y1^T
            tc.tile_pool(name="resS", bufs=1) as resS,   # small persistents
            tc.tile_pool(name="xld", bufs=2) as xld,
            tc.tile_pool(name="wtp", bufs=16) as wtp,
            tc.tile_pool(name="outp", bufs=2) as outp,
            tc.tile_pool(name="gt", bufs=2) as gt,
            tc.tile_pool(name="ps_rp", bufs=2, space="PSUM") as ps_rp,
            tc.tile_pool(name="ps_xp", bufs=2, space="PSUM") as ps_xp,
            tc.tile_pool(name="ps_tp", bufs=2, space="PSUM") as ps_tp,
        ):
            # ---- persistent small tiles ----
            idf = resS.tile([P, P], F32)
            make_identity(nc, idf[:])
            idb = resS.tile([P, P], BF16)
            make_identity(nc, idb[:])
            bx1 = resS.tile([P, MT], F32)
            nc.sync.dma_start(bx1[:], bx1_d[:])
            bx2 = resS.tile([P, MT], F32)
            nc.sync.dma_start(bx2[:], bx2_d[:])
            bh1 = resS.tile([P, KT * BL], F32)
            nc.sync.dma_start(bh1[:], bh1_d[:])
            bh2 = resS.tile([P, KT * BL], F32)
            nc.sync.dma_start(bh2[:], bh2_d[:])
            h1i = resS.tile([P, KT * BL], BF16)
            h2i = resS.tile([P, KT * BL], BF16)
            nc.vector.memset(h2i[:], 0.0)
            y2r = resS.tile([P, 2 * KT * CROWS], BF16)   # layer-2 ring (2 chunks)

            # ---- phase A: U1, hidden transpose, x transpose ----
            u1s = resA.tile([P, KT * G3], BF16, tag="ubig")
            for k in range(KT):
                nc.sync.dma_start(u1s[:, k * G3:(k + 1) * G3], u1_d[k * P:(k + 1) * P, :])

            hid = outp.tile([P, UN], F32, tag="outst")
            nc.sync.dma_start(hid[0:BL, :], hid_d[:])
            for k in range(KT):
                tp = ps_tp.tile([P, P], F32, tag="tp")
                nc.tensor.transpose(tp[:, 0:BL], hid[0:BL, k * P:(k + 1) * P], idf[0:BL, 0:BL])
                nc.vector.tensor_copy(h1i[:, k * BL:(k + 1) * BL], tp[:, 0:BL])

            xts = resC.tile([P, KT * ROWS], BF16, tag="rbig")
            for c in range(NCH):
                xl = xld.tile([P, F], F32, tag="xl")
                nc.sync.dma_start(xl[:], xsrc[c])
                for k in range(KT):
                    tp = ps_tp.tile([P, P], F32, tag="tp")
                    nc.tensor.transpose(tp[:], xl[:, k * P:(k + 1) * P], idf[:])
                    nc.vector.tensor_copy(
                        xts[:, k * ROWS + c * CROWS: k * ROWS + (c + 1) * CROWS], tp[:])

            # ---- xp matmul phase (shared by B and D) ----
            def xp_phase(w_dram, rhs, out, bx):
                for m in range(MT):
                    wts = []
                    for k in range(KT):
                        wt = wtp.tile([P, P], BF16, tag="wt")
                        nc.sync.dma_start(wt[:], w_dram[k * MT + m])
                        wts.append(wt)
                    for n in range(N4):
                        pp = ps_xp.tile([P, NW], F32, tag="xpp")
                        for k in range(KT):
                            nc.tensor.matmul(
                                pp[:], wts[k][:],
                                rhs[:, k * ROWS + n * NW: k * ROWS + (n + 1) * NW],
                                start=(k == 0), stop=(k == KT - 1))
                        nc.scalar.activation(
                            out[:, m * ROWS + n * NW: m * ROWS + (n + 1) * NW],
                            pp[:], AF.Identity, bias=bx[:, m:m + 1])

            # ---- GRU step (shared by C and E) ----
            def gru_step(us, xps, bh, hsrc, hdst_gate, hsrc_gate, xpr):
                """hsrc: k -> [P, BL] AP of h^{t-1}; hdst_gate/hsrc_gate: [P,KT,BL]-like APs."""
                rp = ps_rp.tile([P, MT * BL], F32, tag="rp")
                for m in range(MT):
                    for k in range(KT):
                        nc.tensor.matmul(
                            rp[:, m * BL:(m + 1) * BL],
                            us[:, k * G3 + m * P: k * G3 + (m + 1) * P],
                            hsrc(k), start=(k == 0), stop=(k == KT - 1))
                zrp = gt.tile([P, 2 * KT * BL], F32, tag="zrp")
                nc.vector.tensor_tensor(zrp[:], rp[:, 0:2 * KT * BL], xpr(0, 16), OP.add)
                zrs = gt.tile([P, 2 * KT * BL], F32, tag="zrs")
                nc.scalar.activation(zrs[:], zrp[:], AF.Sigmoid)
                rhp = gt.tile([P, KT * BL], F32, tag="rhp")
                nc.vector.tensor_tensor(rhp[:], rp[:, 2 * KT * BL:], bh[:], OP.add)
                prod = gt.tile([P, KT * BL], F32, tag="prod")
                nc.vector.tensor_tensor(prod[:], zrs[:, KT * BL:], rhp[:], OP.mult)
                preh = gt.tile([P, KT * BL], F32, tag="preh")
                nc.vector.tensor_tensor(preh[:], prod[:], xpr(16, 24), OP.add)
                hh = gt.tile([P, KT * BL], F32, tag="hh")
                nc.scalar.activation(hh[:], preh[:], AF.Tanh)
                d = gt.tile([P, KT * BL], F32, tag="d")
                nc.vector.tensor_tensor(d[:], hsrc_gate, hh[:], OP.subtract)
                zd = gt.tile([P, KT * BL], F32, tag="zd")
                nc.vector.tensor_tensor(zd[:], zrs[:, 0:KT * BL], d[:], OP.mult)
                nc.vector.tensor_tensor(hdst_gate, zd[:], hh[:], OP.add)

            # ---- phase B: xp1 ----
            xp1s = resB.tile([P, MT * ROWS], BF16, tag="xpbig")
            xp_phase(w1_d, xts[:], xp1s[:], bx1)

            # ---- phase C: layer-1 recurrence ----
            y1s = resC.tile([P, KT * ROWS], BF16, tag="rbig")
            y1r = y1s[:].rearrange("p (k r) -> p k r", k=KT)
            h1ir = h1i[:].rearrange("p (k b) -> p k b", k=KT)
            xp1r = xp1s[:].rearrange("p (m r) -> p m r", m=MT)
            for t in range(T):
                if t == 0:
                    hsrc = lambda k: h1i[:, k * BL:(k + 1) * BL]
                    hsg = h1ir[:, :, :]
                else:
                    hsrc = lambda k, t=t: y1s[:, k * ROWS + (t - 1) * BL: k * ROWS + t * BL]
                    hsg = y1r[:, :, (t - 1) * BL: t * BL]
                gru_step(
                    u1s, xp1s, bh1, hsrc,
                    y1r[:, :, t * BL:(t + 1) * BL], hsg,
                    lambda m0, m1, t=t: xp1r[:, m0:m1, t * BL:(t + 1) * BL])

            # ---- phase D: xp2 (U2 load overlaps) ----
            u2s = resA.tile([P, KT * G3], BF16, tag="ubig")
            for k in range(KT):
                nc.sync.dma_start(u2s[:, k * G3:(k + 1) * G3], u2_d[k * P:(k + 1) * P, :])
            xp2s = resB.tile([P, MT * ROWS], BF16, tag="xpbig")
            xp_phase(w2_d, y1s[:], xp2s[:], bx2)

            # ---- phase E: layer-2 recurrence + output ----
            xp2r = xp2s[:].rearrange("p (m r) -> p m r", m=MT)
            y2rr = y2r[:].rearrange("p (q k r) -> p q k r", q=2, k=KT)
            h2ir = h2i[:].rearrange("p (k b) -> p k b", k=KT)
            for t in range(T):
                c, tt = t // CH, t % CH
                par, opar = c % 2, 1 - (c % 2)

                def sec(q, k):
                    return q * KT * CROWS + k * CROWS

                if t == 0:
                    hsrc = lambda k: h2i[:, k * BL:(k + 1) * BL]
                    hsg = h2ir[:, :, :]
                elif tt == 0:
                    hsrc = lambda k, par=opar: y2r[:, sec(par, k) + (CH - 1) * BL: sec(par, k) + CH * BL]
                    hsg = y2rr[:, opar, :, (CH - 1) * BL: CH * BL]
                else:
                    hsrc = lambda k, par=par, tt=tt: y2r[:, sec(par, k) + (tt - 1) * BL: sec(par, k) + tt * BL]
                    hsg = y2rr[:, par, :, (tt - 1) * BL: tt * BL]
                gru_step(
                    u2s, xp2s, bh2, hsrc,
                    y2rr[:, par, :, tt * BL:(tt + 1) * BL], hsg,
                    lambda m0, m1, t=t: xp2r[:, m0:m1, t * BL:(t + 1) * BL])

                if tt == CH - 1:
                    outst = outp.tile([P, UN], F32, tag="outst")
                    for k in range(KT):
                        tpb = ps_tp.tile([P, P], BF16, tag="tp")
                        nc.tensor.transpose(tpb[:], y2r[:, sec(par, k): sec(par, k) + CROWS], idb[:])
                        nc.vector.tensor_copy(outst[:, k * P:(k + 1) * P], tpb[:])
                    nc.gpsimd.dma_start(ydst[c], outst[:])
                    if c == NCH - 1:
                        nc.gpsimd.dma_start(hl_d[:], outst[(CH - 1) * BL:, :])

    nc.compile()
    return nc


def _prep_shared(W1, U1, b1, W2, U2, b2):
    bf = ml_dtypes.bfloat16

    def tile_w(W):
        # [F, G3] -> [(k m), p, c] with tile (k, m) = W[128k:.., 128m:..]
        return np.ascontiguousarray(
            W.astype(bf).reshape(KT, P, MT, P).transpose(0, 2, 1, 3).reshape(KT * MT, P, P))

    def bias_xp(b):
        v = b[0].astype(np.float64) + np.concatenate([b[1][:2 * UN], np.zeros(UN)])
        return np.ascontiguousarray(v.reshape(MT, P).T.astype(np.float32))

    def bias_h(b):
        v = b[1][2 * UN:].astype(np.float32).reshape(KT, P).T  # [P, KT]
        return np.ascontiguousarray(np.repeat(v[:, :, None], BL, axis=2).reshape(P, KT * BL))

    return {
        "u1": np.ascontiguousarray(U1.astype(bf)),
        "u2": np.ascontiguousarray(U2.astype(bf)),
        "w1t": tile_w(W1), "w2t": tile_w(W2),
        "bx1": bias_xp(b1), "bx2": bias_xp(b2),
        "bh1": bias_h(b1), "bh2": bias_h(b2),
    }


def kernel(x, hidden, W1, U1, b1, W2, U2, b2):
    x = np.asarray(x, dtype=np.float32)
    hidden = np.asarray(hidden, dtype=np.float32)
    W1, U1, b1 = np.asarray(W1), np.asarray(U1), np.asarray(b1)
    W2, U2, b2 = np.asarray(W2), np.asarray(U2), np.asarray(b2)

    if "nc" not in _CACHED:
        _CACHED["nc"] = _build()
    nc = _CACHED["nc"]

    shared = _prep_shared(W1, U1, b1, W2, U2, b2)
    in_maps = []
    for c in range(NCORES):
        m = dict(shared)
        m["x"] = np.ascontiguousarray(x[c * BL:(c + 1) * BL])
        m["hidden"] = np.ascontiguousarray(hidden[c * BL:(c + 1) * BL])
        in_maps.append(m)

    res = run_bass_kernel_spmd(nc, in_maps, core_ids=list(range(NCORES)))
    y = np.concatenate([res.results[c]["y"] for c in range(NCORES)], axis=0)
    state = np.concatenate([res.results[c]["hlast"] for c in range(NCORES)], axis=0)
    return (y, state)
